# revision 1
# baseline (speedup 1.0000x reference)
"""Kendall-tau loss kernel for Trainium2 (Bass/Tile), 8-core SPMD.

Math (per row, N=2048, no ties in this fixed input):
  After sorting target by pred order, tau = (conc-disc)/(conc+disc).
  With no ties conc+disc = P = N(N-1)/2 and
    conc - disc = S/2,  S = sum_{a!=b} sign(p_b-p_a)*sign(t_b-t_a)
  so tau = S / (N(N-1)) and no sorting is needed at all -- S is a pure
  O(N^2) pairwise computation.

  Counting: over ALL ordered pairs (a, b),
    sum [t_b > t_a] * sign(p_b - p_a) = conc - disc
  (each unordered pair contributes exactly once, in its t-ascending
  orientation: +1 concordant, -1 discordant), so tau = (conc-disc)/P.

Device work per 128-element a-chunk (a on partitions, all b on free):
  - ScalarE: sp = Sign(p_broadcast + bias(-p_a))          [128, 2048]
  - VectorE: scalar_tensor_tensor((t_broadcast is_gt t_a) mult sp,
             accum_out) -> per-partition (conc-disc) partial
  - GPSIMD:  only negates the per-chunk scalar columns
  The DVE pass is the critical path (~35us/row); ACT sign production
  (~30us/row) overlaps it under the Tile scheduler.

  NOTE this container's walrus rejects >1 sem-wait per instruction and
  cannot encode custom-DVE ISA ops at all; see _patch_tile_drain and
  _split_waits (the registered custom op in _register_op is unused).

Sharding: 32 rows (B*T) data-parallel, 4 rows per core; scalar
reduction of the Q-counts happens on host (tiny).
"""

import os
import numpy as np
from operator import add

N = 2048
P = 128
NCHUNK = N // P  # 16
ROWS_PER_CORE = 4
N_CORES = 8
COLS_PER_ROW = NCHUNK  # one conc-count column per chunk
NUP = 128 * sum(N - P * (c + 1) for c in range(NCHUNK - 1))  # 1966080
NDIAG = NCHUNK * P * (P - 1)  # 260096

_OP_NAME = "KTAU_PAIRCOUNT_ANT"
_cache = {}


def _register_op():
    """Create + register the fused pair-count DVE op (idempotent)."""
    import concourse.dve_ops as dve_ops

    for op in dve_ops.OPS:
        if op.name == _OP_NAME:
            return op

    from concourse.dve_spec import (
        Spec,
        Src0,
        Src1,
        C0,
        C1,
        Zero,
        lower as dve_lower,
        _has_src1,
    )
    from concourse.dve_uop import DveOpSpec

    def _ref(in0, in1, s0, s1, imm2):
        s0 = np.asarray(s0, np.float32).reshape(-1, 1)
        s1 = np.asarray(s1, np.float32).reshape(-1, 1)
        b = (
            ((in0.astype(np.float32) - s0) * (in1.astype(np.float32) - s1)) > 0
        ).astype(np.float32)
        return b, b.reshape(b.shape[0], -1).sum(axis=-1, keepdims=True)

    spec = Spec(
        body=((Src0 - C0) * (Src1 - C1)) > Zero,
        accum=add,
        accum_init=Zero,
        reference=_ref,
    )
    row = 1 + len(dve_ops.OPS)
    assert row < 0x20
    dve_ops._SUB_OPCODE_FOR_NAME[_OP_NAME] = row
    shas = {}
    for ver in ("v3", "v4"):
        uops = dve_lower(spec, ver=ver)
        shas[ver] = DveOpSpec(
            name=_OP_NAME, opcode=row, uops=uops, rd1_en=_has_src1(spec)
        ).sha(ver)
    op = dve_ops.DveOp(_OP_NAME, spec, subdim=False, uops_sha=shas)
    dve_ops.OPS.append(op)
    dve_ops.CUSTOM_DVE_SPECS[_OP_NAME] = spec
    return op


def _patch_tile_drain():
    """The walrus build in this container rejects sync-waits on CTRL
    instructions (Drain/NOP): "Too many sync wait commands" for any
    wait count >= 1.  Replace TileContext's kernel-tail drain-with-waits
    by an equivalent chain of event-semaphore wait_ge instructions
    (which this compiler encodes fine) followed by a bare drain."""
    import concourse.mybir as mybir
    from concourse.tile import TileContext, ScopedClock

    if getattr(TileContext, "_ktau_drain_patched", False):
        return

    def _drain_and_barrier(self, tick_clock, wait_clock):
        tmp = self.nc.sync.nop()
        wait_clock.add_sem_waits(
            tmp.ins, ScopedClock({None: tick_clock.global_clock})
        )
        waits = list(tmp.ins.sync_info.on_wait)
        tmp.ins.sync_info = mybir.SyncInfo(
            on_update=list(tmp.ins.sync_info.on_update), on_wait=[]
        )
        num2handle = {h.num: h for h in self.sems.allocated().values()}
        for w in waits:
            self.nc.sync.wait_ge(num2handle[w.id], w.wait_value)
        self.nc.sync.drain()
        self.nc.all_engine_barrier()
        popped = self.nc._tile_sem_poison_stack.pop()
        assert popped is self._sem_poison
        self.nc.clear_and_free_semaphores(list(self.sems.allocated().values()))
        self.nc.all_engine_barrier()

    TileContext._drain_and_barrier = _drain_and_barrier
    TileContext._ktau_drain_patched = True


def _split_waits(nc, max_waits=1):
    """This container's walrus encodes at most one sem-wait per
    instruction ("Too many sync wait commands" / "ISA wrong length"
    otherwise).  Hoist excess waits onto single-wait EventSemaphore
    instructions inserted just before the consumer on the same engine
    (engines execute their stream in order, so semantics are identical)."""
    import concourse.mybir as mybir

    n = 0
    for fn in nc.m.functions:
        for bb in fn.blocks:
            new_list = []
            for ins in bb.instructions:
                si = ins.sync_info
                waits = list(si.on_wait) if si is not None else []
                if len(waits) > max_waits:
                    for w in waits[:-max_waits]:
                        n += 1
                        ev = mybir.InstEventSemaphore(
                            name=f"WSPLIT-{n}",
                            engine=ins.engine,
                            sync_info=mybir.SyncInfo(on_update=[], on_wait=[w]),
                        )
                        new_list.append(ev)
                    ins.sync_info = mybir.SyncInfo(
                        on_update=list(si.on_update), on_wait=waits[-max_waits:]
                    )
                new_list.append(ins)
            bb.instructions = new_list


def _build_nc():
    import concourse.bass as bass
    import concourse.mybir as mybir
    import concourse.tile as tile

    op = _register_op()
    _patch_tile_drain()
    f32 = mybir.dt.float32
    bf16 = mybir.dt.bfloat16

    nc = bass.Bass("TRN2")
    p_in = nc.dram_tensor("p", [ROWS_PER_CORE, N], f32, kind="ExternalInput")
    t_in = nc.dram_tensor("t", [ROWS_PER_CORE, N], f32, kind="ExternalInput")
    q_out = nc.dram_tensor(
        "q", [P, ROWS_PER_CORE * COLS_PER_ROW], f32, kind="ExternalOutput"
    )

    with tile.TileContext(nc) as tc:
        with (
            tc.tile_pool(name="bcast", bufs=2) as bpool,
            tc.tile_pool(name="cols", bufs=2) as cpool,
            tc.tile_pool(name="scr", bufs=4) as spool,
            tc.tile_pool(name="acc", bufs=1) as apool,
        ):
            qacc = apool.tile([P, ROWS_PER_CORE * COLS_PER_ROW], f32)
            for r in range(ROWS_PER_CORE):
                pb = bpool.tile([P, N], f32, tag="pb")
                tb = bpool.tile([P, N], f32, tag="tb")
                nc.sync.dma_start(pb[:], p_in[r : r + 1, :].to_broadcast((P, N)))
                nc.sync.dma_start(tb[:], t_in[r : r + 1, :].to_broadcast((P, N)))
                # p_cols[i, c] = p[128c + i]  (chunk c of the row on free dim c)
                pc = cpool.tile([P, NCHUNK], f32, tag="pc")
                tcl = cpool.tile([P, NCHUNK], f32, tag="tc")
                nc.sync.dma_start(
                    pc[:], p_in[r, :].rearrange("(c p) -> p c", p=P)
                )
                nc.sync.dma_start(
                    tcl[:], t_in[r, :].rearrange("(c p) -> p c", p=P)
                )
                npc = cpool.tile([P, NCHUNK], f32, tag="npc")
                nc.gpsimd.tensor_scalar(
                    npc[:], pc[:], -1.0, None, mybir.AluOpType.mult
                )
                base = r * NCHUNK
                for c in range(NCHUNK):
                    # full tile: a in chunk c (partitions) vs ALL b (free).
                    # sp = sign(p_b - p_a) on ScalarE; the DVE pass sums
                    # [t_b > t_a] * sp, which counts each unordered pair once
                    # (its t-ascending orientation): +1 concordant,
                    # -1 discordant => accum = conc - disc for this a-chunk.
                    sp = spool.tile([P, N], f32, tag="sp")
                    nc.scalar.activation(
                        sp[:], pb[:],
                        mybir.ActivationFunctionType.Sign,
                        bias=npc[:, c : c + 1], scale=1.0,
                    )
                    scr = spool.tile([P, N], f32, tag="scr")
                    nc.vector.scalar_tensor_tensor(
                        scr[:],
                        tb[:],
                        tcl[:, c : c + 1],
                        sp[:],
                        mybir.AluOpType.is_gt,
                        mybir.AluOpType.mult,
                        accum_out=qacc[:, base + c : base + c + 1],
                    )
            nc.sync.dma_start(q_out[:], qacc[:])
    _split_waits(nc)
    return nc


def _get_nc():
    if "nc" not in _cache:
        _cache["nc"] = _build_nc()
    return _cache["nc"]


def kernel(pred, target):
    from concourse.bass_utils import run_bass_kernel_spmd

    pred = np.ascontiguousarray(np.asarray(pred, dtype=np.float32)).reshape(-1, N)
    target = np.ascontiguousarray(np.asarray(target, dtype=np.float32)).reshape(-1, N)
    n_rows = pred.shape[0]
    assert n_rows == ROWS_PER_CORE * N_CORES

    nc = _get_nc()
    in_maps = [
        {
            "p": np.ascontiguousarray(pred[k * ROWS_PER_CORE : (k + 1) * ROWS_PER_CORE]),
            "t": np.ascontiguousarray(target[k * ROWS_PER_CORE : (k + 1) * ROWS_PER_CORE]),
        }
        for k in range(N_CORES)
    ]
    trace = bool(int(os.environ.get("KTAU_TRACE", "0")))
    try:
        res = run_bass_kernel_spmd(
            nc,
            in_maps,
            core_ids=list(range(N_CORES)),
            trace=trace,
            **({"trace_cores": list(range(N_CORES)), "stitch_traces": True} if trace else {}),
        )
    except ModuleNotFoundError:
        # NTFF profiling hook unavailable in this container -- run untraced.
        res = run_bass_kernel_spmd(nc, in_maps, core_ids=list(range(N_CORES)))
    _cache["last_perf"] = res

    q = np.stack([r["q"] for r in res.results]).astype(np.float64)  # [8,128,64]
    s_total = q.sum()  # sum over rows of (conc - disc)
    pairs = float(N * (N - 1) // 2)  # conc+disc per row (no ties)
    # tau_row = (conc-disc)/pairs; loss = 1 - mean(tau_row)
    loss = 1.0 - s_total / (n_rows * pairs)
    return np.float32(loss)



# revision 4
# speedup vs baseline: 5.7783x; 5.7783x over previous
"""Kendall-tau loss kernel for Trainium2 (Bass/Tile), 8-core SPMD.

Math (per row, N=2048): reference sorts target by pred order (stable
argsort) and counts concordant/discordant pairs over positions i<j:
  tau = (conc - disc) / (conc + disc),  loss = 1 - mean(tau).

Host does the O(N log N) argsort of pred (tiny: 32x2048) and ships
ta = target[argsort(pred)] to the device; the O(N^2) pair counting
runs on-device:
  conc - disc = sum_{i<j} sign(ta_j - ta_i)
which handles pred-ties exactly like the reference (stable order ->
pair counted by t-order) and target-ties exactly (sign(0)=0, and the
denominator drops t-tied pairs: conc+disc = P - Tt, corrected on host).

Device work per 128-element a-chunk c (a on partitions, b on free):
  - off-diag (b in chunks > c): ScalarE activation(Sign, bias=-ta_a,
    accum_out) -> per-partition sum of sign(ta_b - ta_a). Exact.
  - diag (b in chunk c): VectorE scalar_tensor_tensor
    ((tb is_gt ta_a) mult mask2, accum_out) where mask2 = 2.0 on b>=a
    -> per-partition 2*#(b>a, ta_b > ta_a).
  Per-row accumulator columns are reduced on-device (DVE tensor_reduce)
  to [128, 4]; host sums partitions. Device output is 2KB/core, so the
  device->host fetch over the axon tunnel is one small batched read.
Host combines: C_r = S_offdiag_r + 2*cnt_diag_r, so
  conc - disc = C_r - DIAG_PAIRS + ties_diag_r  (host counts the few
  t-tie pairs and which fall inside a diagonal chunk).

Counts stay < 2^24 so f32 accumulation is exact.

Sharding: 32 rows (B*T) data-parallel, 4 rows per core, per the
"trivially data-parallel" hint; the final scalar mean happens on host
(the all-reduce of 8 scalars).

Transport: the container is chipless; devices are reached through the
axon PJRT tunnel with ~50-100ms round-trip latency. A fresh
jax.jit(shard_map(...)) per call (what run_bass_kernel_spmd ->
run_bass_via_pjrt does) costs several round-trips, so we hoist that
exact lowering (same _bass_exec_p path run_bass_kernel_spmd uses under
axon) and cache the jitted callable: warm calls are one async dispatch
plus one batched result fetch = one round-trip.

NOTE this container's walrus rejects >1 sem-wait per instruction; see
_patch_tile_drain and _split_waits.
"""

import numpy as np

N = 2048
P = 128
NCHUNK = N // P  # 16
ROWS_PER_CORE = 4
N_CORES = 8
NROWS = ROWS_PER_CORE * N_CORES  # 32
COLS_PER_ROW = 32  # 16 diag + 15 off-diag + 1 pad
PAIRS = N * (N - 1) // 2  # 2096128
DIAG_PAIRS = NCHUNK * (P * (P - 1) // 2)  # 130048

_cache = {}


def _patch_tile_drain():
    """The walrus build in this container rejects sync-waits on CTRL
    instructions (Drain/NOP): "Too many sync wait commands" for any
    wait count >= 1.  Replace TileContext's kernel-tail drain-with-waits
    by an equivalent chain of event-semaphore wait_ge instructions
    (which this compiler encodes fine) followed by a bare drain."""
    import concourse.mybir as mybir
    from concourse.tile import TileContext, ScopedClock

    if getattr(TileContext, "_ktau_drain_patched", False):
        return

    def _drain_and_barrier(self, tick_clock, wait_clock):
        tmp = self.nc.sync.nop()
        wait_clock.add_sem_waits(
            tmp.ins, ScopedClock({None: tick_clock.global_clock})
        )
        waits = list(tmp.ins.sync_info.on_wait)
        tmp.ins.sync_info = mybir.SyncInfo(
            on_update=list(tmp.ins.sync_info.on_update), on_wait=[]
        )
        num2handle = {h.num: h for h in self.sems.allocated().values()}
        for w in waits:
            self.nc.sync.wait_ge(num2handle[w.id], w.wait_value)
        self.nc.sync.drain()
        self.nc.all_engine_barrier()
        popped = self.nc._tile_sem_poison_stack.pop()
        assert popped is self._sem_poison
        self.nc.clear_and_free_semaphores(list(self.sems.allocated().values()))
        self.nc.all_engine_barrier()

    TileContext._drain_and_barrier = _drain_and_barrier
    TileContext._ktau_drain_patched = True


def _split_waits(nc, max_waits=1):
    """This container's walrus encodes at most one sem-wait per
    instruction ("Too many sync wait commands" / "ISA wrong length"
    otherwise).  Hoist excess waits onto single-wait EventSemaphore
    instructions inserted just before the consumer on the same engine
    (engines execute their stream in order, so semantics are identical)."""
    import concourse.mybir as mybir

    n = 0
    for fn in nc.m.functions:
        for bb in fn.blocks:
            new_list = []
            for ins in bb.instructions:
                si = ins.sync_info
                waits = list(si.on_wait) if si is not None else []
                if len(waits) > max_waits:
                    for w in waits[:-max_waits]:
                        n += 1
                        ev = mybir.InstEventSemaphore(
                            name=f"WSPLIT-{n}",
                            engine=ins.engine,
                            sync_info=mybir.SyncInfo(on_update=[], on_wait=[w]),
                        )
                        new_list.append(ev)
                    ins.sync_info = mybir.SyncInfo(
                        on_update=list(si.on_update), on_wait=waits[-max_waits:]
                    )
                new_list.append(ins)
            bb.instructions = new_list


def _build_nc(split_waits=True):
    import concourse.bass as bass
    import concourse.mybir as mybir
    import concourse.tile as tile
    from concourse.masks import make_upper_triangular

    _patch_tile_drain()
    f32 = mybir.dt.float32

    nc = bass.Bass("TRN2")
    t_in = nc.dram_tensor("t", [ROWS_PER_CORE, N], f32, kind="ExternalInput")
    q_out = nc.dram_tensor("q", [P, ROWS_PER_CORE], f32, kind="ExternalOutput")

    with tile.TileContext(nc) as tc:
        with (
            tc.tile_pool(name="bcast", bufs=2) as bpool,
            tc.tile_pool(name="cols", bufs=2) as cpool,
            tc.tile_pool(name="sa", bufs=3) as sapool,
            tc.tile_pool(name="sd", bufs=2) as sdpool,
            tc.tile_pool(name="acc", bufs=1) as apool,
        ):
            mask2 = apool.tile([P, P], f32)
            make_upper_triangular(nc, mask2[:], val=2.0, diag=True)
            qacc = apool.tile([P, ROWS_PER_CORE * COLS_PER_ROW], f32)
            nc.gpsimd.memset(qacc[:], 0.0)
            qred = apool.tile([P, ROWS_PER_CORE], f32)
            for r in range(ROWS_PER_CORE):
                tb = bpool.tile([P, N], f32, tag="tb")
                nc.sync.dma_start(tb[:], t_in[r : r + 1, :].to_broadcast((P, N)))
                # tcl[p, c] = ta[128c + p] (chunk c of the row on free dim c)
                tcl = cpool.tile([P, NCHUNK], f32, tag="tc")
                nc.sync.dma_start(
                    tcl[:], t_in[r, :].rearrange("(c p) -> p c", p=P)
                )
                ntc = cpool.tile([P, NCHUNK], f32, tag="ntc")
                nc.gpsimd.tensor_scalar(
                    ntc[:], tcl[:], -1.0, None, mybir.AluOpType.mult
                )
                base = r * COLS_PER_ROW
                for c in range(NCHUNK):
                    # diag chunk: a = 128c + p on partitions, b = same chunk
                    # on free. accum = 2 * #(b > a with ta_b > ta_a).
                    scr = sdpool.tile([P, P], f32, tag="sd")
                    nc.vector.scalar_tensor_tensor(
                        scr[:],
                        tb[:, c * P : (c + 1) * P],
                        tcl[:, c : c + 1],
                        mask2[:],
                        mybir.AluOpType.is_gt,
                        mybir.AluOpType.mult,
                        accum_out=qacc[:, base + c : base + c + 1],
                    )
                for c in range(NCHUNK - 1):
                    # off-diag: b over all chunks > c. accum = per-partition
                    # sum of sign(ta_b - ta_a). Exact (ties -> 0).
                    w = N - (c + 1) * P
                    scr = sapool.tile([P, N - P], f32, tag="sa")
                    nc.scalar.activation(
                        scr[:, :w],
                        tb[:, (c + 1) * P :],
                        mybir.ActivationFunctionType.Sign,
                        bias=ntc[:, c : c + 1],
                        scale=1.0,
                        accum_out=qacc[:, base + NCHUNK + c : base + NCHUNK + c + 1],
                    )
            for r in range(ROWS_PER_CORE):
                nc.vector.tensor_reduce(
                    qred[:, r : r + 1],
                    qacc[:, r * COLS_PER_ROW : (r + 1) * COLS_PER_ROW],
                    mybir.AxisListType.X,
                    mybir.AluOpType.add,
                )
            nc.sync.dma_start(q_out[:], qred[:])
    if split_waits:
        _split_waits(nc)
    return nc


def _get_runner():
    """Build the Bass module once and cache a jitted SPMD callable.

    This is the same lowering run_bass_kernel_spmd performs under axon
    (bass2jax.run_bass_via_pjrt), hoisted so the jax.jit(shard_map(...))
    wrapper -- and therefore the XLA/NEFF compile -- happens once per
    process instead of once per call."""
    if "runner" in _cache:
        return _cache["runner"]

    import jax
    import jax.core
    from jax.experimental.shard_map import shard_map
    from jax.sharding import Mesh, PartitionSpec

    import concourse.mybir as mybir
    from concourse.bass2jax import (
        _bass_exec_p,
        install_neuronx_cc_hook,
        partition_id_tensor,
    )

    nc = _build_nc()
    install_neuronx_cc_hook()
    partition_name = nc.partition_id_tensor.name if nc.partition_id_tensor else None

    in_names, out_names, out_avals, zero_outs = [], [], [], []
    for alloc in nc.m.functions[0].allocations:
        if not isinstance(alloc, mybir.MemoryLocationSet):
            continue
        name = alloc.memorylocations[0].name
        if alloc.kind == "ExternalInput":
            if name != partition_name:
                in_names.append(name)
        elif alloc.kind == "ExternalOutput":
            shape = tuple(alloc.tensor_shape)
            dtype = mybir.dt.np(alloc.dtype)
            out_names.append(name)
            out_avals.append(jax.core.ShapedArray(shape, dtype))
            zero_outs.append(np.zeros(shape, dtype))
    n_params = len(in_names)
    n_outs = len(out_avals)
    all_in_names = list(in_names) + list(out_names)
    if partition_name is not None:
        all_in_names.append(partition_name)
    donate = tuple(range(n_params, n_params + n_outs))

    def _body(*args):
        operands = list(args)
        if partition_name is not None:
            operands.append(partition_id_tensor())
        outs = _bass_exec_p.bind(
            *operands,
            out_avals=tuple(out_avals),
            in_names=tuple(all_in_names),
            out_names=tuple(out_names),
            lowering_input_output_aliases=(),
            sim_require_finite=True,
            sim_require_nnan=True,
            nc=nc,
        )
        return tuple(outs)

    devices = jax.devices()[:N_CORES]
    assert len(devices) == N_CORES
    mesh = Mesh(np.asarray(devices), ("core",))
    in_specs = (PartitionSpec("core"),) * (n_params + n_outs)
    out_specs = (PartitionSpec("core"),) * n_outs
    sharded = jax.jit(
        shard_map(
            _body, mesh=mesh, in_specs=in_specs, out_specs=out_specs, check_rep=False
        ),
        donate_argnums=donate,
        keep_unused=True,
    )

    def run(per_core_t):
        # per_core_t: [N_CORES, ROWS_PER_CORE, N] f32 -> [N_CORES, P, ROWS_PER_CORE]
        concat_in = np.ascontiguousarray(
            per_core_t.reshape(N_CORES * ROWS_PER_CORE, N)
        )
        concat_zeros = [
            np.zeros((N_CORES * z.shape[0], *z.shape[1:]), z.dtype) for z in zero_outs
        ]
        out_arrs = sharded(concat_in, *concat_zeros)
        return np.asarray(out_arrs[0]).reshape(N_CORES, P, ROWS_PER_CORE)

    _cache["runner"] = run
    return run


def _tie_stats(ta):
    """Per row of ta [NROWS, N]: (# t-tied pairs, # t-tied pairs whose
    positions fall in the same 128-aligned diagonal chunk)."""
    from itertools import combinations

    tt = np.zeros(ta.shape[0], np.int64)
    td = np.zeros(ta.shape[0], np.int64)
    for r in range(ta.shape[0]):
        row = ta[r]
        order = np.argsort(row, kind="stable")
        sval = row[order]
        run_start = 0
        for i in range(1, N + 1):
            if i == N or sval[i] != sval[run_start]:
                k = i - run_start
                if k > 1:
                    pos = order[run_start:i]
                    tt[r] += k * (k - 1) // 2
                    for a, b in combinations(pos, 2):
                        if a // P == b // P:
                            td[r] += 1
                run_start = i
    return tt, td


def kernel(pred, target):
    pred = np.ascontiguousarray(np.asarray(pred, dtype=np.float32)).reshape(-1, N)
    target = np.ascontiguousarray(np.asarray(target, dtype=np.float32)).reshape(-1, N)
    assert pred.shape[0] == NROWS

    # Host presort: ta = target in pred-ascending (stable) order, matching
    # the reference's jnp.argsort (stable) exactly.
    idx = np.argsort(pred, axis=1, kind="stable")
    ta = np.take_along_axis(target, idx, axis=1)

    run = _get_runner()
    q = run(ta.reshape(N_CORES, ROWS_PER_CORE, N))  # [8, 128, 4]
    _cache["last_q"] = q

    # C_r = S_offdiag_r + 2*cnt_diag_r per row
    C = q.astype(np.float64).sum(axis=1).reshape(NROWS)
    tt, td = _tie_stats(ta)
    s = C - DIAG_PAIRS + td  # conc - disc
    denom = PAIRS - tt       # conc + disc
    tau = s / denom
    loss = 1.0 - tau.mean()
    return np.float32(loss)


# revision 6
# speedup vs baseline: 8.1061x; 1.4029x over previous
"""Kendall-tau loss kernel for Trainium2 (Bass/Tile), 8-core SPMD.

Math (per row, N=2048): reference sorts target by pred order (stable
argsort) and counts concordant/discordant pairs over positions i<j:
  tau = (conc - disc) / (conc + disc),  loss = 1 - mean(tau).

Host does the O(N log N) argsort of pred (tiny: 32x2048) and ships
ta = target[argsort(pred)] to the device; the O(N^2) pair counting
runs on-device:
  conc - disc = sum_{i<j} sign(ta_j - ta_i)
which handles pred-ties exactly like the reference (stable order ->
pair counted by t-order) and target-ties exactly (sign(0)=0, and the
denominator drops t-tied pairs: conc+disc = P - Tt, corrected on host).

Device work per 128-element a-chunk c (a on partitions, b on free):
  - off-diag (b in chunks > c): ScalarE activation(Sign, bias=-ta_a,
    accum_out) -> per-partition sum of sign(ta_b - ta_a). Exact.
  - diag (b in chunk c): VectorE scalar_tensor_tensor
    ((tb is_gt ta_a) mult mask2, accum_out) where mask2 = 2.0 on b>=a
    -> per-partition 2*#(b>a, ta_b > ta_a).
  Per-row accumulator columns are reduced on-device (DVE tensor_reduce)
  to [128, 4]; host sums partitions. Device output is 2KB/core, so the
  device->host fetch over the axon tunnel is one small batched read.
Host combines: C_r = S_offdiag_r + 2*cnt_diag_r, so
  conc - disc = C_r - DIAG_PAIRS + ties_diag_r  (host counts the few
  t-tie pairs and which fall inside a diagonal chunk).

Counts stay < 2^24 so f32 accumulation is exact.

Sharding: 32 rows (B*T) data-parallel, 4 rows per core, per the
"trivially data-parallel" hint; the final scalar mean happens on host
(the all-reduce of 8 scalars).

Transport: the container is chipless; devices are reached through the
axon PJRT tunnel with ~50-100ms round-trip latency. A fresh
jax.jit(shard_map(...)) per call (what run_bass_kernel_spmd ->
run_bass_via_pjrt does) costs several round-trips, so we hoist that
exact lowering (same _bass_exec_p path run_bass_kernel_spmd uses under
axon) and cache the jitted callable: warm calls are one async dispatch
plus one batched result fetch = one round-trip.

NOTE this container's walrus rejects >1 sem-wait per instruction; see
_patch_tile_drain and _split_waits.
"""

import numpy as np

N = 2048
P = 128
NCHUNK = N // P  # 16
ROWS_PER_CORE = 4
N_CORES = 8
NROWS = ROWS_PER_CORE * N_CORES  # 32
COLS_PER_ROW = 32  # 16 diag + 15 off-diag + 1 pad
PAIRS = N * (N - 1) // 2  # 2096128
DIAG_PAIRS = NCHUNK * (P * (P - 1) // 2)  # 130048

_cache = {}


def _patch_tile_drain():
    """The walrus build in this container rejects sync-waits on CTRL
    instructions (Drain/NOP): "Too many sync wait commands" for any
    wait count >= 1.  Replace TileContext's kernel-tail drain-with-waits
    by an equivalent chain of event-semaphore wait_ge instructions
    (which this compiler encodes fine) followed by a bare drain."""
    import concourse.mybir as mybir
    from concourse.tile import TileContext, ScopedClock

    if getattr(TileContext, "_ktau_drain_patched", False):
        return

    def _drain_and_barrier(self, tick_clock, wait_clock):
        tmp = self.nc.sync.nop()
        wait_clock.add_sem_waits(
            tmp.ins, ScopedClock({None: tick_clock.global_clock})
        )
        waits = list(tmp.ins.sync_info.on_wait)
        tmp.ins.sync_info = mybir.SyncInfo(
            on_update=list(tmp.ins.sync_info.on_update), on_wait=[]
        )
        num2handle = {h.num: h for h in self.sems.allocated().values()}
        for w in waits:
            self.nc.sync.wait_ge(num2handle[w.id], w.wait_value)
        self.nc.sync.drain()
        self.nc.all_engine_barrier()
        popped = self.nc._tile_sem_poison_stack.pop()
        assert popped is self._sem_poison
        self.nc.clear_and_free_semaphores(list(self.sems.allocated().values()))
        self.nc.all_engine_barrier()

    TileContext._drain_and_barrier = _drain_and_barrier
    TileContext._ktau_drain_patched = True


def _split_waits(nc, max_waits=1):
    """This container's walrus encodes at most one sem-wait per
    instruction ("Too many sync wait commands" / "ISA wrong length"
    otherwise).  Hoist excess waits onto single-wait EventSemaphore
    instructions inserted just before the consumer on the same engine
    (engines execute their stream in order, so semantics are identical)."""
    import concourse.mybir as mybir

    n = 0
    for fn in nc.m.functions:
        for bb in fn.blocks:
            new_list = []
            for ins in bb.instructions:
                si = ins.sync_info
                waits = list(si.on_wait) if si is not None else []
                if len(waits) > max_waits:
                    for w in waits[:-max_waits]:
                        n += 1
                        ev = mybir.InstEventSemaphore(
                            name=f"WSPLIT-{n}",
                            engine=ins.engine,
                            sync_info=mybir.SyncInfo(on_update=[], on_wait=[w]),
                        )
                        new_list.append(ev)
                    ins.sync_info = mybir.SyncInfo(
                        on_update=list(si.on_update), on_wait=waits[-max_waits:]
                    )
                new_list.append(ins)
            bb.instructions = new_list


def _build_nc(split_waits=True):
    import concourse.bass as bass
    import concourse.mybir as mybir
    import concourse.tile as tile
    from concourse.masks import make_upper_triangular

    _patch_tile_drain()
    f32 = mybir.dt.float32

    nc = bass.Bass("TRN2")
    t_in = nc.dram_tensor("t", [ROWS_PER_CORE, N], f32, kind="ExternalInput")
    q_out = nc.dram_tensor("q", [P, ROWS_PER_CORE], f32, kind="ExternalOutput")

    with tile.TileContext(nc) as tc:
        with (
            tc.tile_pool(name="bcast", bufs=2) as bpool,
            tc.tile_pool(name="cols", bufs=2) as cpool,
            tc.tile_pool(name="sa", bufs=3) as sapool,
            tc.tile_pool(name="sd", bufs=2) as sdpool,
            tc.tile_pool(name="acc", bufs=1) as apool,
        ):
            mask2 = apool.tile([P, P], f32)
            make_upper_triangular(nc, mask2[:], val=2.0, diag=True)
            qacc = apool.tile([P, ROWS_PER_CORE * COLS_PER_ROW], f32)
            nc.gpsimd.memset(qacc[:], 0.0)
            qred = apool.tile([P, ROWS_PER_CORE], f32)
            for r in range(ROWS_PER_CORE):
                tb = bpool.tile([P, N], f32, tag="tb")
                nc.sync.dma_start(tb[:], t_in[r : r + 1, :].to_broadcast((P, N)))
                # tcl[p, c] = ta[128c + p] (chunk c of the row on free dim c)
                tcl = cpool.tile([P, NCHUNK], f32, tag="tc")
                nc.sync.dma_start(
                    tcl[:], t_in[r, :].rearrange("(c p) -> p c", p=P)
                )
                ntc = cpool.tile([P, NCHUNK], f32, tag="ntc")
                nc.gpsimd.tensor_scalar(
                    ntc[:], tcl[:], -1.0, None, mybir.AluOpType.mult
                )
                base = r * COLS_PER_ROW
                for c in range(NCHUNK):
                    # diag chunk: a = 128c + p on partitions, b = same chunk
                    # on free. accum = 2 * #(b > a with ta_b > ta_a).
                    scr = sdpool.tile([P, P], f32, tag="sd")
                    nc.vector.scalar_tensor_tensor(
                        scr[:],
                        tb[:, c * P : (c + 1) * P],
                        tcl[:, c : c + 1],
                        mask2[:],
                        mybir.AluOpType.is_gt,
                        mybir.AluOpType.mult,
                        accum_out=qacc[:, base + c : base + c + 1],
                    )
                for c in range(NCHUNK - 1):
                    # off-diag: b over all chunks > c. accum = per-partition
                    # sum of sign(ta_b - ta_a). Exact (ties -> 0).
                    w = N - (c + 1) * P
                    scr = sapool.tile([P, N - P], f32, tag="sa")
                    nc.scalar.activation(
                        scr[:, :w],
                        tb[:, (c + 1) * P :],
                        mybir.ActivationFunctionType.Sign,
                        bias=ntc[:, c : c + 1],
                        scale=1.0,
                        accum_out=qacc[:, base + NCHUNK + c : base + NCHUNK + c + 1],
                    )
            for r in range(ROWS_PER_CORE):
                nc.vector.tensor_reduce(
                    qred[:, r : r + 1],
                    qacc[:, r * COLS_PER_ROW : (r + 1) * COLS_PER_ROW],
                    mybir.AxisListType.X,
                    mybir.AluOpType.add,
                )
            nc.sync.dma_start(q_out[:], qred[:])
    if split_waits:
        _split_waits(nc)
    return nc


def _get_runner():
    """Build the Bass module once and cache a jitted SPMD callable.

    This is the same lowering run_bass_kernel_spmd performs under axon
    (bass2jax.run_bass_via_pjrt), hoisted so the jax.jit(shard_map(...))
    wrapper -- and therefore the XLA/NEFF compile -- happens once per
    process instead of once per call."""
    if "runner" in _cache:
        return _cache["runner"]

    import jax
    import jax.core
    from jax.experimental.shard_map import shard_map
    from jax.sharding import Mesh, PartitionSpec

    import concourse.mybir as mybir
    from concourse.bass2jax import (
        _bass_exec_p,
        install_neuronx_cc_hook,
        partition_id_tensor,
    )

    nc = _build_nc()
    install_neuronx_cc_hook()
    partition_name = nc.partition_id_tensor.name if nc.partition_id_tensor else None

    in_names, out_names, out_avals, zero_outs = [], [], [], []
    for alloc in nc.m.functions[0].allocations:
        if not isinstance(alloc, mybir.MemoryLocationSet):
            continue
        name = alloc.memorylocations[0].name
        if alloc.kind == "ExternalInput":
            if name != partition_name:
                in_names.append(name)
        elif alloc.kind == "ExternalOutput":
            shape = tuple(alloc.tensor_shape)
            dtype = mybir.dt.np(alloc.dtype)
            out_names.append(name)
            out_avals.append(jax.core.ShapedArray(shape, dtype))
            zero_outs.append(np.zeros(shape, dtype))
    n_params = len(in_names)
    n_outs = len(out_avals)
    all_in_names = list(in_names) + list(out_names)
    if partition_name is not None:
        all_in_names.append(partition_name)
    donate = tuple(range(n_params, n_params + n_outs))

    def _body(*args):
        operands = list(args)
        if partition_name is not None:
            operands.append(partition_id_tensor())
        outs = _bass_exec_p.bind(
            *operands,
            out_avals=tuple(out_avals),
            in_names=tuple(all_in_names),
            out_names=tuple(out_names),
            lowering_input_output_aliases=(),
            sim_require_finite=True,
            sim_require_nnan=True,
            nc=nc,
        )
        return tuple(outs)

    devices = jax.devices()[:N_CORES]
    assert len(devices) == N_CORES
    mesh = Mesh(np.asarray(devices), ("core",))
    in_specs = (PartitionSpec("core"),) * (n_params + n_outs)
    out_specs = (PartitionSpec("core"),) * n_outs
    sharded = jax.jit(
        shard_map(
            _body, mesh=mesh, in_specs=in_specs, out_specs=out_specs, check_rep=False
        ),
        donate_argnums=donate,
        keep_unused=True,
    )

    def run(per_core_t):
        # per_core_t: [N_CORES, ROWS_PER_CORE, N] f32 -> [N_CORES, P, ROWS_PER_CORE]
        concat_in = np.ascontiguousarray(
            per_core_t.reshape(N_CORES * ROWS_PER_CORE, N)
        )
        concat_zeros = [
            np.zeros((N_CORES * z.shape[0], *z.shape[1:]), z.dtype) for z in zero_outs
        ]
        out_arrs = sharded(concat_in, *concat_zeros)
        return np.asarray(out_arrs[0]).reshape(N_CORES, P, ROWS_PER_CORE)

    _cache["runner"] = run
    return run


def _tie_stats(ta):
    """Per row of ta [NROWS, N]: (# t-tied pairs, # t-tied pairs whose
    positions fall in the same 128-aligned diagonal chunk).

    Exact ties are rare (a handful across all rows), so scan a sorted
    copy for adjacent equals (vectorized) and only walk the few rows
    that have any."""
    from itertools import combinations

    tt = np.zeros(ta.shape[0], np.int64)
    td = np.zeros(ta.shape[0], np.int64)
    sv = np.sort(ta, axis=1)
    has = (sv[:, 1:] == sv[:, :-1]).any(axis=1)
    for r in np.nonzero(has)[0]:
        row = ta[r]
        order = np.argsort(row, kind="stable")
        sval = row[order]
        e = sval[1:] == sval[:-1]
        i = 0
        while i < N - 1:
            if e[i]:
                j = i
                while j < N - 1 and e[j]:
                    j += 1
                pos = order[i : j + 1]
                k = j + 1 - i
                tt[r] += k * (k - 1) // 2
                for a, b in combinations(pos, 2):
                    if a // P == b // P:
                        td[r] += 1
                i = j + 1
            else:
                i += 1
    return tt, td


def _stable_pred_argsort(pred):
    """Stable ascending argsort of each row, matching jnp.argsort.

    Quicksort is ~6x faster than a stable sort here; stability only
    matters for exactly-equal pred values (rare), so fix those groups
    up to ascending original index afterwards."""
    idx = np.argsort(pred, axis=1, kind="quicksort")
    pv = np.take_along_axis(pred, idx, axis=1)
    eqrows = (pv[:, 1:] == pv[:, :-1]).any(axis=1)
    for r in np.nonzero(eqrows)[0]:
        e = pv[r, 1:] == pv[r, :-1]
        i = 0
        while i < N - 1:
            if e[i]:
                j = i
                while j < N - 1 and e[j]:
                    j += 1
                idx[r, i : j + 1] = np.sort(idx[r, i : j + 1])
                i = j + 1
            else:
                i += 1
    return idx


def kernel(pred, target):
    pred = np.ascontiguousarray(np.asarray(pred, dtype=np.float32)).reshape(-1, N)
    target = np.ascontiguousarray(np.asarray(target, dtype=np.float32)).reshape(-1, N)
    assert pred.shape[0] == NROWS

    # Host presort: ta = target in pred-ascending (stable) order, matching
    # the reference's jnp.argsort (stable) exactly.
    idx = _stable_pred_argsort(pred)
    ta = np.take_along_axis(target, idx, axis=1)

    run = _get_runner()
    q = run(ta.reshape(N_CORES, ROWS_PER_CORE, N))  # [8, 128, 4]
    _cache["last_q"] = q

    # C_r = S_offdiag_r + 2*cnt_diag_r per row
    C = q.astype(np.float64).sum(axis=1).reshape(NROWS)
    tt, td = _tie_stats(ta)
    s = C - DIAG_PAIRS + td  # conc - disc
    denom = PAIRS - tt       # conc + disc
    tau = s / denom
    loss = 1.0 - tau.mean()
    return np.float32(loss)


# revision 9
# speedup vs baseline: 8.8865x; 1.0963x over previous
"""Kendall-tau loss kernel for Trainium2 (Bass/Tile), 8-core SPMD.

Math (per row, N=2048): reference sorts target by pred order (stable
argsort) and counts concordant/discordant pairs over positions i<j:
  tau = (conc - disc) / (conc + disc),  loss = 1 - mean(tau).

Host does the O(N log N) argsort of pred (tiny: 32x2048) and ships
ta = target[argsort(pred)] to the device; the O(N^2) pair counting
runs on-device:
  conc - disc = sum_{i<j} sign(ta_j - ta_i)
which handles pred-ties exactly like the reference (stable order ->
pair counted by t-order) and target-ties exactly (sign(0)=0, and the
denominator drops t-tied pairs: conc+disc = P - Tt, corrected on host).

Device work per 128-element a-chunk c (a on partitions, b on free):
  - off-diag (b in chunks > c): ScalarE activation(Sign, bias=-ta_a,
    accum_out) -> per-partition sum of sign(ta_b - ta_a). Exact.
  - diag (b in chunk c): VectorE scalar_tensor_tensor
    ((tb is_gt ta_a) mult mask2, accum_out) where mask2 = 2.0 on b>=a
    -> per-partition 2*#(b>a, ta_b > ta_a).
  Per-row accumulator columns are reduced on-device (DVE tensor_reduce)
  to [128, 4]; host sums partitions. Device output is 2KB/core, so the
  device->host fetch over the axon tunnel is one small batched read.
Host combines: C_r = S_offdiag_r + 2*cnt_diag_r, so
  conc - disc = C_r - DIAG_PAIRS + ties_diag_r  (host counts the few
  t-tie pairs and which fall inside a diagonal chunk).

Counts stay < 2^24 so f32 accumulation is exact.

Sharding: 32 rows (B*T) data-parallel, 4 rows per core, per the
"trivially data-parallel" hint; the final scalar mean happens on host
(the all-reduce of 8 scalars).

Transport: the container is chipless; devices are reached through the
axon PJRT tunnel with ~50-100ms round-trip latency. A fresh
jax.jit(shard_map(...)) per call (what run_bass_kernel_spmd ->
run_bass_via_pjrt does) costs several round-trips, so we hoist that
exact lowering (same _bass_exec_p path run_bass_kernel_spmd uses under
axon) and cache the jitted callable: warm calls are one async dispatch
plus one batched result fetch = one round-trip.

NOTE this container's walrus rejects >1 sem-wait per instruction; see
_patch_tile_drain and _split_waits.
"""

import numpy as np

N = 2048
P = 128
NCHUNK = N // P  # 16
ROWS_PER_CORE = 4
N_CORES = 8
NROWS = ROWS_PER_CORE * N_CORES  # 32
COLS_PER_ROW = 32  # 16 diag + 15 off-diag + 1 pad
PAIRS = N * (N - 1) // 2  # 2096128
DIAG_PAIRS = NCHUNK * (P * (P - 1) // 2)  # 130048

_cache = {}


def _patch_tile_drain():
    """The walrus build in this container rejects sync-waits on CTRL
    instructions (Drain/NOP): "Too many sync wait commands" for any
    wait count >= 1.  Replace TileContext's kernel-tail drain-with-waits
    by an equivalent chain of event-semaphore wait_ge instructions
    (which this compiler encodes fine) followed by a bare drain."""
    import concourse.mybir as mybir
    from concourse.tile import TileContext, ScopedClock

    if getattr(TileContext, "_ktau_drain_patched", False):
        return

    def _drain_and_barrier(self, tick_clock, wait_clock):
        tmp = self.nc.sync.nop()
        wait_clock.add_sem_waits(
            tmp.ins, ScopedClock({None: tick_clock.global_clock})
        )
        waits = list(tmp.ins.sync_info.on_wait)
        tmp.ins.sync_info = mybir.SyncInfo(
            on_update=list(tmp.ins.sync_info.on_update), on_wait=[]
        )
        num2handle = {h.num: h for h in self.sems.allocated().values()}
        for w in waits:
            self.nc.sync.wait_ge(num2handle[w.id], w.wait_value)
        self.nc.sync.drain()
        self.nc.all_engine_barrier()
        popped = self.nc._tile_sem_poison_stack.pop()
        assert popped is self._sem_poison
        self.nc.clear_and_free_semaphores(list(self.sems.allocated().values()))
        self.nc.all_engine_barrier()

    TileContext._drain_and_barrier = _drain_and_barrier
    TileContext._ktau_drain_patched = True


def _split_waits(nc, max_waits=1):
    """This container's walrus encodes at most one sem-wait per
    instruction ("Too many sync wait commands" / "ISA wrong length"
    otherwise).  Hoist excess waits onto single-wait EventSemaphore
    instructions inserted just before the consumer on the same engine
    (engines execute their stream in order, so semantics are identical)."""
    import concourse.mybir as mybir

    n = 0
    for fn in nc.m.functions:
        for bb in fn.blocks:
            new_list = []
            for ins in bb.instructions:
                si = ins.sync_info
                waits = list(si.on_wait) if si is not None else []
                if len(waits) > max_waits:
                    for w in waits[:-max_waits]:
                        n += 1
                        ev = mybir.InstEventSemaphore(
                            name=f"WSPLIT-{n}",
                            engine=ins.engine,
                            sync_info=mybir.SyncInfo(on_update=[], on_wait=[w]),
                        )
                        new_list.append(ev)
                    ins.sync_info = mybir.SyncInfo(
                        on_update=list(si.on_update), on_wait=waits[-max_waits:]
                    )
                new_list.append(ins)
            bb.instructions = new_list


def _build_nc(split_waits=True):
    import concourse.bass as bass
    import concourse.mybir as mybir
    import concourse.tile as tile
    from concourse.masks import make_upper_triangular

    _patch_tile_drain()
    f32 = mybir.dt.float32

    nc = bass.Bass("TRN2")
    t_in = nc.dram_tensor("t", [ROWS_PER_CORE, N], f32, kind="ExternalInput")
    q_out = nc.dram_tensor("q", [P, ROWS_PER_CORE], f32, kind="ExternalOutput")

    with tile.TileContext(nc) as tc:
        with (
            tc.tile_pool(name="bcast", bufs=2) as bpool,
            tc.tile_pool(name="cols", bufs=2) as cpool,
            tc.tile_pool(name="sa", bufs=3) as sapool,
            tc.tile_pool(name="sd", bufs=2) as sdpool,
            tc.tile_pool(name="acc", bufs=1) as apool,
        ):
            mask2 = apool.tile([P, P], f32)
            make_upper_triangular(nc, mask2[:], val=2.0, diag=True)
            qacc = apool.tile([P, ROWS_PER_CORE * COLS_PER_ROW], f32)
            nc.gpsimd.memset(qacc[:], 0.0)
            qred = apool.tile([P, ROWS_PER_CORE], f32)
            for r in range(ROWS_PER_CORE):
                tb = bpool.tile([P, N], f32, tag="tb")
                nc.sync.dma_start(tb[:], t_in[r : r + 1, :].to_broadcast((P, N)))
                # tcl[p, c] = ta[128c + p] (chunk c of the row on free dim c)
                tcl = cpool.tile([P, NCHUNK], f32, tag="tc")
                nc.sync.dma_start(
                    tcl[:], t_in[r, :].rearrange("(c p) -> p c", p=P)
                )
                ntc = cpool.tile([P, NCHUNK], f32, tag="ntc")
                nc.gpsimd.tensor_scalar(
                    ntc[:], tcl[:], -1.0, None, mybir.AluOpType.mult
                )
                base = r * COLS_PER_ROW
                for c in range(NCHUNK):
                    # diag chunk: a = 128c + p on partitions, b = same chunk
                    # on free. accum = 2 * #(b > a with ta_b > ta_a).
                    scr = sdpool.tile([P, P], f32, tag="sd")
                    nc.vector.scalar_tensor_tensor(
                        scr[:],
                        tb[:, c * P : (c + 1) * P],
                        tcl[:, c : c + 1],
                        mask2[:],
                        mybir.AluOpType.is_gt,
                        mybir.AluOpType.mult,
                        accum_out=qacc[:, base + c : base + c + 1],
                    )
                for c in range(NCHUNK - 1):
                    # off-diag: b over all chunks > c. accum = per-partition
                    # sum of sign(ta_b - ta_a). Exact (ties -> 0).
                    w = N - (c + 1) * P
                    scr = sapool.tile([P, N - P], f32, tag="sa")
                    nc.scalar.activation(
                        scr[:, :w],
                        tb[:, (c + 1) * P :],
                        mybir.ActivationFunctionType.Sign,
                        bias=ntc[:, c : c + 1],
                        scale=1.0,
                        accum_out=qacc[:, base + NCHUNK + c : base + NCHUNK + c + 1],
                    )
            for r in range(ROWS_PER_CORE):
                nc.vector.tensor_reduce(
                    qred[:, r : r + 1],
                    qacc[:, r * COLS_PER_ROW : (r + 1) * COLS_PER_ROW],
                    mybir.AxisListType.X,
                    mybir.AluOpType.add,
                )
            nc.sync.dma_start(q_out[:], qred[:])
    if split_waits:
        _split_waits(nc)
    return nc


def _get_runner():
    """Build the Bass module once and cache a jitted SPMD callable.

    This is the same lowering run_bass_kernel_spmd performs under axon
    (bass2jax.run_bass_via_pjrt), hoisted so the jax.jit(shard_map(...))
    wrapper -- and therefore the XLA/NEFF compile -- happens once per
    process instead of once per call."""
    if "runner" in _cache:
        return _cache["runner"]

    import jax
    import jax.core
    from jax.experimental.shard_map import shard_map
    from jax.sharding import Mesh, PartitionSpec

    # Persistent compilation cache: if the PJRT plugin supports executable
    # serialization this makes the cold call in a fresh process skip the
    # minutes-long walrus BIR->NEFF compile. Harmless no-op otherwise.
    try:
        jax.config.update("jax_compilation_cache_dir", "/root/.cache/jax-ktau-cache")
        jax.config.update("jax_persistent_cache_min_compile_time_secs", 1.0)
        jax.config.update("jax_persistent_cache_min_entry_size_bytes", 0)
    except Exception:
        pass

    import concourse.mybir as mybir
    from concourse.bass2jax import (
        _bass_exec_p,
        install_neuronx_cc_hook,
        partition_id_tensor,
    )

    nc = _build_nc()
    install_neuronx_cc_hook()
    partition_name = nc.partition_id_tensor.name if nc.partition_id_tensor else None

    in_names, out_names, out_avals, zero_outs = [], [], [], []
    for alloc in nc.m.functions[0].allocations:
        if not isinstance(alloc, mybir.MemoryLocationSet):
            continue
        name = alloc.memorylocations[0].name
        if alloc.kind == "ExternalInput":
            if name != partition_name:
                in_names.append(name)
        elif alloc.kind == "ExternalOutput":
            shape = tuple(alloc.tensor_shape)
            dtype = mybir.dt.np(alloc.dtype)
            out_names.append(name)
            out_avals.append(jax.core.ShapedArray(shape, dtype))
            zero_outs.append(np.zeros(shape, dtype))
    n_params = len(in_names)
    n_outs = len(out_avals)
    all_in_names = list(in_names) + list(out_names)
    if partition_name is not None:
        all_in_names.append(partition_name)
    donate = tuple(range(n_params, n_params + n_outs))

    def _body(*args):
        operands = list(args)
        if partition_name is not None:
            operands.append(partition_id_tensor())
        outs = _bass_exec_p.bind(
            *operands,
            out_avals=tuple(out_avals),
            in_names=tuple(all_in_names),
            out_names=tuple(out_names),
            lowering_input_output_aliases=(),
            sim_require_finite=True,
            sim_require_nnan=True,
            nc=nc,
        )
        return tuple(outs)

    devices = jax.devices()[:N_CORES]
    assert len(devices) == N_CORES
    mesh = Mesh(np.asarray(devices), ("core",))
    in_specs = (PartitionSpec("core"),) * (n_params + n_outs)
    out_specs = (PartitionSpec("core"),) * n_outs
    sharded = jax.jit(
        shard_map(
            _body, mesh=mesh, in_specs=in_specs, out_specs=out_specs, check_rep=False
        ),
        donate_argnums=donate,
        keep_unused=True,
    )

    def dispatch(per_core_t):
        # per_core_t: [N_CORES, ROWS_PER_CORE, N] f32 -> async jax arrays
        concat_in = np.ascontiguousarray(
            per_core_t.reshape(N_CORES * ROWS_PER_CORE, N)
        )
        concat_zeros = [
            np.zeros((N_CORES * z.shape[0], *z.shape[1:]), z.dtype) for z in zero_outs
        ]
        return sharded(concat_in, *concat_zeros)

    def collect(out_arrs):
        # -> [N_CORES, P, ROWS_PER_CORE]
        return np.asarray(out_arrs[0]).reshape(N_CORES, P, ROWS_PER_CORE)

    _cache["runner"] = (dispatch, collect)
    return _cache["runner"]


def _tie_stats(ta):
    """Per row of ta [NROWS, N]: (# t-tied pairs, # t-tied pairs whose
    positions fall in the same 128-aligned diagonal chunk).

    Exact ties are rare (a handful across all rows), so scan a sorted
    copy for adjacent equals (vectorized) and only walk the few rows
    that have any."""
    from itertools import combinations

    tt = np.zeros(ta.shape[0], np.int64)
    td = np.zeros(ta.shape[0], np.int64)
    sv = np.sort(ta, axis=1)
    has = (sv[:, 1:] == sv[:, :-1]).any(axis=1)
    for r in np.nonzero(has)[0]:
        row = ta[r]
        order = np.argsort(row, kind="stable")
        sval = row[order]
        e = sval[1:] == sval[:-1]
        i = 0
        while i < N - 1:
            if e[i]:
                j = i
                while j < N - 1 and e[j]:
                    j += 1
                pos = order[i : j + 1]
                k = j + 1 - i
                tt[r] += k * (k - 1) // 2
                for a, b in combinations(pos, 2):
                    if a // P == b // P:
                        td[r] += 1
                i = j + 1
            else:
                i += 1
    return tt, td


def _stable_pred_argsort(pred):
    """Stable ascending argsort of each row, matching jnp.argsort.

    Quicksort is ~6x faster than a stable sort here; stability only
    matters for exactly-equal pred values (rare), so fix those groups
    up to ascending original index afterwards."""
    idx = np.argsort(pred, axis=1, kind="quicksort")
    pv = np.take_along_axis(pred, idx, axis=1)
    eqrows = (pv[:, 1:] == pv[:, :-1]).any(axis=1)
    for r in np.nonzero(eqrows)[0]:
        e = pv[r, 1:] == pv[r, :-1]
        i = 0
        while i < N - 1:
            if e[i]:
                j = i
                while j < N - 1 and e[j]:
                    j += 1
                idx[r, i : j + 1] = np.sort(idx[r, i : j + 1])
                i = j + 1
            else:
                i += 1
    return idx


def kernel(pred, target):
    pred = np.ascontiguousarray(np.asarray(pred, dtype=np.float32)).reshape(-1, N)
    target = np.ascontiguousarray(np.asarray(target, dtype=np.float32)).reshape(-1, N)
    assert pred.shape[0] == NROWS

    # Host presort: ta = target in pred-ascending (stable) order, matching
    # the reference's jnp.argsort (stable) exactly.
    idx = _stable_pred_argsort(pred)
    ta = np.take_along_axis(target, idx, axis=1)

    dispatch, collect = _get_runner()
    out = dispatch(ta.reshape(N_CORES, ROWS_PER_CORE, N))
    # Host tie counting overlaps the device round-trip.
    tt, td = _tie_stats(ta)
    q = collect(out)  # [8, 128, 4]
    _cache["last_q"] = q

    # C_r = S_offdiag_r + 2*cnt_diag_r per row
    C = q.astype(np.float64).sum(axis=1).reshape(NROWS)
    s = C - DIAG_PAIRS + td  # conc - disc
    denom = PAIRS - tt       # conc + disc
    tau = s / denom
    loss = 1.0 - tau.mean()
    return np.float32(loss)
